# revision 22
# baseline (speedup 1.0000x reference)
"""Trainium2 Bass kernel for nn_Attention (B=2, S=2048, D=2048, H=16, hd=128).

Sharding: 2-way batch DP x 4-way head TP over 8 cores.
Core c: batch b = c//4, head-group g = c%4 (heads 4g..4g+4).

Per-core pipeline (single SPMD program, per-core behavior via input data only):
  Weights wq/wk/wv ([D, 512] each, f16) are SBUF-resident (loaded once,
  interleaved with the first x^T stream so the PE starts within ~2us).
  Per s-quarter q (512 rows):
    Phase 1: QKV projections from pre-transposed x (x^T in HBM), RoPE applied
             to Q^T/K^T in [hd, S] layout (ACT evacuates PSUM to f16, DVE
             does the 16-bit rotates). The hd axis of q/k weights is
             host-permuted (even indices first) so RoPE pairs become
             partition halves (i, 64+i); scores are invariant to a shared
             q/k hd-perm.
    Phase 2: causal attention for q-tile q, all heads: scores computed
             TRANSPOSED (sT[k, q-tile] = K^T.T @ Q^T), exp on ACT, diagonal
             blocks zeroed post-exp with a {0,1} f16 mask on DVE, row-sums
             via ones-matmul, PV matmul -> attn^T[hd, q], normalized by 1/l
             broadcast (K=1 ones matmul).
  AllGather attn^T shards within each batch group of 4 cores. The cost is
  latency-dominated (~15us/collective regardless of size), so quarters 0-2
  use ONE 4-head gather each; the last quarter uses two 2-head gathers so
  its out-proj only waits on the last pair.
  Phase 3: out-proj slice: out[:, 512 cols of this group] from gathered
  attn^T (single descriptor-efficient reload per gather).

All matmuls use f16 operands with f32 PSUM accumulation.

build(reps=N) emits the whole body N times in one NEFF (collectives cannot
sit inside control flow). test.py uses this to measure the marginal
per-execution device time, which per-dispatch axon RPC overhead otherwise
hides completely.
"""

import math
import sys

import numpy as np

for _p in ("/opt/trn_rl_repo",):
    if _p not in sys.path:
        sys.path.insert(0, _p)

import concourse.bass as bass
import concourse.mybir as mybir
from concourse import bacc
from concourse.tile import TileContext

B, S, D, H, HD = 2, 2048, 2048, 16, 128
NC_TOTAL = 8
TPG = 4                 # head-TP group size
HPC = H // TPG          # heads per core = 4
P = 128
NDC = D // P            # 16 contraction chunks
ST = 512                # s/q tile width
NST = S // ST           # 4

f32 = mybir.dt.float32
f32r = mybir.dt.float32r
f16 = mybir.dt.float16
AF = mybir.ActivationFunctionType
ALU = mybir.AluOpType

_NC_CACHE = {}

# gather layout used by kernel() and test.py (see build() docstring)
GMODE = 0


def build(sim_single_core: bool = False, null_kernel: bool = False,
          reps: int = 1, gmode: int = 0) -> bass.Bass:
    nc = bacc.Bacc("TRN2", target_bir_lowering=False, debug=False,
                   num_devices=NC_TOTAL)

    xt = nc.declare_dram_parameter("xt", [D, S], f16, isOutput=False)
    wq_t = nc.declare_dram_parameter("wq_t", [D, HPC * HD], f16, isOutput=False)
    wk_t = nc.declare_dram_parameter("wk_t", [D, HPC * HD], f16, isOutput=False)
    wv_t = nc.declare_dram_parameter("wv_t", [D, HPC * HD], f16, isOutput=False)
    wo_t = nc.declare_dram_parameter("wo_t", [D, ST], f16, isOutput=False)
    cs_lo = nc.declare_dram_parameter("cs_lo", [64, S], f16, isOutput=False)
    sn_ng = nc.declare_dram_parameter("sn_ng", [64, S], f16, isOutput=False)
    mb = nc.declare_dram_parameter("mb", [P, 4, ST], f16, isOutput=False)
    out = nc.declare_dram_parameter("out", [S, ST], f32, isOutput=True)

    if null_kernel:
        with TileContext(nc) as tc:
            with (
                tc.tile_pool(name="sb", bufs=1) as sb,
                tc.tile_pool(name="dram", bufs=1, space="DRAM") as dpool,
            ):
                cc_in = dpool.tile([HPC * HD, ST], f16)
                cc_out = dpool.tile([D, ST], f16)
                t = sb.tile([P, ST], f16)
                nc.sync.dma_start(t[:], xt[0:P, 0:ST])
                nc.sync.dma_start(cc_in[0:P, :], t[:])
                nc.gpsimd.collective_compute(
                    "AllGather", ALU.bypass,
                    replica_groups=[[0, 1, 2, 3], [4, 5, 6, 7]],
                    ins=[cc_in[:]], outs=[cc_out[:]])
                t2 = sb.tile([P, ST], f32)
                nc.vector.tensor_copy(t2[:], t[:])
                nc.sync.dma_start(out[0:P, :], t2[:])
        nc.compile()
        return nc

    RG = [[0, 1, 2, 3], [4, 5, 6, 7]]

    with TileContext(nc) as tc:
        with (
            tc.tile_pool(name="const", bufs=1) as cpool,
            tc.tile_pool(name="big", bufs=1) as big,
            tc.tile_pool(name="ps", bufs=1, space="PSUM") as ps,
            tc.tile_pool(name="dram", bufs=1, space="DRAM") as dpool,
        ):
            # ---- persistent SBUF (per-partition KB in comments) ----
            mb_sb = cpool.tile([P, 4, ST], f16)               # 4K {0,1} mask
            ones_col = cpool.tile([P, 1], f16)
            ones_row = cpool.tile([1, P], f32r)
            ones_f = cpool.tile([P, 1], f32)
            onesr_f = cpool.tile([1, P], f32)
            wo_sb = cpool.tile([P, NDC, ST], f16)             # 16K
            wq_sb = cpool.tile([P, NDC, HPC * HD], f16)       # 16K
            wk_sb = cpool.tile([P, NDC, HPC * HD], f16)       # 16K
            wv_sb = cpool.tile([P, NDC, HPC * HD], f16)       # 16K
            nc.vector.memset(ones_f[:], 1.0)
            nc.vector.memset(onesr_f[:], 1.0)
            nc.vector.tensor_copy(ones_col[:], ones_f[:])
            nc.vector.tensor_copy(ones_row[:], onesr_f[:])

            kt_all = big.tile([P, HPC, S], f16)               # 16K
            v_all = big.tile([P, S // P, HPC * HD], f16)      # 16K

            # collective DRAM buffers. gmode=0: quarters 0-2 use ONE
            # 4-head gather (good when cost is latency-dominated); quarter 3
            # uses two 2-head gathers so its out-proj only waits on the last
            # pair. gmode=1: every quarter uses two 2-head gathers launched
            # as soon as each head pair is done (good when cost is
            # volume-dominated and overlaps compute).
            n_pairq = NST if gmode == 1 else 1
            n_singq = NST if gmode == 5 else NST - 1
            cc_in_q = [dpool.tile([HPC * HD, ST], f16, name=f"cc_in{j}")
                       for j in range(n_singq)]
            cc_out_q = [dpool.tile([D, ST], f16, name=f"cc_out{j}")
                        for j in range(n_singq)]
            cc3_in = [[dpool.tile([2 * HD, ST], f16, name=f"cc3_in{j}_{p}")
                       for p in range(2)] for j in range(n_pairq)]
            cc3_out = [[dpool.tile([D // 2, ST], f16, name=f"cc3_out{j}_{p}")
                        for p in range(2)] for j in range(n_pairq)]
            # gmode>=2: batch quarters into few big gathers (real HW has a
            # large per-collective latency floor, so fewer gathers win).
            if gmode == 2:
                gplan = [(0, 1), (2, 3)]
            elif gmode == 3:
                gplan = [(0, 1, 2, 3)]
            else:
                gplan = None
            if gplan:
                q2g = {q: gi for gi, grp in enumerate(gplan) for q in grp}
                ccg_in = [dpool.tile([HPC * HD, len(grp) * ST], f16,
                                     name=f"ccg_in{gi}")
                          for gi, grp in enumerate(gplan)]
                ccg_out = [dpool.tile([D, len(grp) * ST], f16,
                                      name=f"ccg_out{gi}")
                           for gi, grp in enumerate(gplan)]

            with tc.tile_pool(name="p12", bufs=1) as p12:

                def emit_rep():
                    def rope_from_psum(dst, qk_ps, cs_q, sn_q):
                        """RoPE in [hd, ST] layout; pairs are partitions
                        (i, 64+i). ACT evacuates PSUM to f16; rotates on DVE
                        at 16-bit rate. cs_q/sn_q are [128, ST] with the
                        table duplicated on both partition halves (SBUF
                        tensor_tensor needs equal base partitions)."""
                        qk_sb = p12.tile([P, ST], f16, tag="rqk", bufs=2,
                                         name="rqk")
                        nc.scalar.activation(qk_sb[:], qk_ps[:], AF.Copy)
                        a_t = p12.tile([64, ST], f16, tag="rt", bufs=3,
                                       name="rt_a")
                        u_t = p12.tile([64, ST], f16, tag="rt", bufs=3,
                                       name="rt_u")
                        nc.vector.tensor_tensor(
                            a_t[:], qk_sb[0:64, :], cs_q[0:64, :], ALU.mult)
                        nc.vector.tensor_tensor(
                            u_t[:], qk_sb[64:128, :], sn_q[64:128, :],
                            ALU.mult)
                        nc.vector.tensor_tensor(
                            dst[0:64, :], a_t[:], u_t[:], ALU.add)
                        a_b = p12.tile([64, ST], f16, tag="rt", bufs=3,
                                       name="rt_ab")
                        u_b = p12.tile([64, ST], f16, tag="rt", bufs=3,
                                       name="rt_ub")
                        nc.vector.tensor_tensor(
                            a_b[:], qk_sb[64:128, :], cs_q[64:128, :],
                            ALU.mult)
                        nc.vector.tensor_tensor(
                            u_b[:], qk_sb[0:64, :], sn_q[0:64, :], ALU.mult)
                        nc.vector.tensor_tensor(
                            dst[64:128, :], a_b[:], u_b[:], ALU.subtract)

                    for q in range(NST):
                        s0 = q * ST
                        # ---------- phase 1 (s-quarter q) ----------
                        xt_q = p12.tile([P, NDC, ST], f16, tag="xtq", bufs=2,
                                        name="xt_q")
                        cs_q = p12.tile([P, ST], f16, tag="csq", bufs=2,
                                        name="cs_q")
                        sn_q = p12.tile([P, ST], f16, tag="snq", bufs=2,
                                        name="sn_q")
                        if q > 0:
                            for hf in range(2):
                                nc.sync.dma_start(
                                    cs_q[64 * hf:64 * (hf + 1), :],
                                    cs_lo[:, s0:s0 + ST])
                                nc.sync.dma_start(
                                    sn_q[64 * hf:64 * (hf + 1), :],
                                    sn_ng[:, s0:s0 + ST])
                        if q == 0:
                            # interleave resident-weight streaming with the
                            # first x^T chunks so V matmuls start immediately
                            # and later weights land just in time.
                            for dg in range(2):
                                nc.sync.dma_start(
                                    xt_q[:, dg * 2:(dg + 1) * 2, :],
                                    xt[dg * 2 * P:(dg + 1) * 2 * P,
                                       s0:s0 + ST].rearrange(
                                        "(o p) s -> p o s", p=P))
                            for wg in range(4):
                                nc.sync.dma_start(
                                    wv_sb[:, wg * 4:(wg + 1) * 4, :],
                                    wv_t[wg * 4 * P:(wg + 1) * 4 * P, :]
                                    .rearrange("(o p) f -> p o f", p=P))
                                if wg < 3:
                                    dg = 2 + 2 * wg
                                    for d2 in range(dg, dg + 2):
                                        nc.sync.dma_start(
                                            xt_q[:, d2 * 2:(d2 + 1) * 2, :],
                                            xt[d2 * 2 * P:(d2 + 1) * 2 * P,
                                               s0:s0 + ST].rearrange(
                                                "(o p) s -> p o s", p=P))
                            for hf in range(2):
                                nc.sync.dma_start(
                                    cs_q[64 * hf:64 * (hf + 1), :],
                                    cs_lo[:, s0:s0 + ST])
                                nc.sync.dma_start(
                                    sn_q[64 * hf:64 * (hf + 1), :],
                                    sn_ng[:, s0:s0 + ST])
                            for wg in range(4):
                                nc.sync.dma_start(
                                    wq_sb[:, wg * 4:(wg + 1) * 4, :],
                                    wq_t[wg * 4 * P:(wg + 1) * 4 * P, :]
                                    .rearrange("(o p) f -> p o f", p=P))
                            for wg in range(4):
                                nc.sync.dma_start(
                                    wk_sb[:, wg * 4:(wg + 1) * 4, :],
                                    wk_t[wg * 4 * P:(wg + 1) * 4 * P, :]
                                    .rearrange("(o p) f -> p o f", p=P))
                            nc.sync.dma_start(mb_sb[:], mb[:])
                            for wg in range(4):
                                nc.sync.dma_start(
                                    wo_sb[:, wg * 4:(wg + 1) * 4, :],
                                    wo_t[wg * 4 * P:(wg + 1) * 4 * P, :]
                                    .rearrange("(o p) f -> p o f", p=P))
                        else:
                            for dg in range(8):
                                nc.sync.dma_start(
                                    xt_q[:, dg * 2:(dg + 1) * 2, :],
                                    xt[dg * 2 * P:(dg + 1) * 2 * P,
                                       s0:s0 + ST].rearrange(
                                        "(o p) s -> p o s", p=P))

                        # V for the 4 s-chunks of this quarter
                        for vs in range(2):
                            v_ps = [
                                ps.tile([P, HPC * HD], f32, tag="vps", bufs=2,
                                        name=f"vps_{q}_{vs}_{i}")
                                for i in range(2)
                            ]
                            for dc in range(NDC):
                                for i in range(2):
                                    sc = vs * 2 + i
                                    nc.tensor.matmul(
                                        v_ps[i][:],
                                        xt_q[:, dc, sc * P:(sc + 1) * P],
                                        wv_sb[:, dc, :],
                                        start=(dc == 0), stop=(dc == NDC - 1),
                                        skip_group_check=True,
                                    )
                            for i in range(2):
                                nc.scalar.activation(
                                    v_all[:, q * 4 + vs * 2 + i, :],
                                    v_ps[i][:], AF.Copy)

                        # Q^T / K^T for this quarter with RoPE
                        qt_q = p12.tile([P, HPC, ST], f16, tag="qtq", bufs=2,
                                        name="qt_q")
                        for h in range(HPC):
                            qt_ps = ps.tile([P, ST], f32, tag="qk", bufs=3,
                                            name="qt_ps")
                            for dc in range(NDC):
                                nc.tensor.matmul(
                                    qt_ps[:], wq_sb[:, dc, h * HD:(h + 1) * HD],
                                    xt_q[:, dc, :],
                                    start=(dc == 0), stop=(dc == NDC - 1),
                                    skip_group_check=True,
                                )
                            rope_from_psum(qt_q[:, h, :], qt_ps, cs_q, sn_q)
                            kt_ps = ps.tile([P, ST], f32, tag="qk", bufs=3,
                                            name="kt_ps")
                            for dc in range(NDC):
                                nc.tensor.matmul(
                                    kt_ps[:], wk_sb[:, dc, h * HD:(h + 1) * HD],
                                    xt_q[:, dc, :],
                                    start=(dc == 0), stop=(dc == NDC - 1),
                                    skip_group_check=True,
                                )
                            rope_from_psum(kt_all[:, h, s0:s0 + ST], kt_ps,
                                           cs_q, sn_q)

                        # ---------- phase 2 (q-tile q, all heads) ----------
                        kcs = 4 * q + 4      # causal: key chunks 0..kcs-1
                        for h in range(HPC):
                            l_ps = ps.tile([1, ST], f32, tag="lob", bufs=2,
                                           name="l_ps")
                            o_ps = ps.tile([P, ST], f32, tag="lob", bufs=2,
                                           name="o_ps")
                            prev_pt = None
                            for kc in range(kcs):
                                st_ps = ps.tile([P, ST], f32, tag="qk", bufs=3,
                                                name="st_ps")
                                nc.tensor.matmul(
                                    st_ps[:],
                                    kt_all[:, h, kc * P:(kc + 1) * P],
                                    qt_q[:, h, :],
                                    start=True, stop=True,
                                    skip_group_check=True,
                                )
                                pt_sb = p12.tile([P, ST], f16, tag="pt",
                                                 bufs=6, name="pt_sb")
                                nc.scalar.activation(pt_sb[:], st_ps[:], AF.Exp)
                                if kc >= 4 * q:   # diagonal block: zero the
                                    # strictly-upper triangle post-exp
                                    nc.vector.tensor_tensor(
                                        pt_sb[:], pt_sb[:],
                                        mb_sb[:, kc - 4 * q, :], ALU.mult)
                                # 4-way tree PT reduction on DVE; the PE
                                # ones-matmul runs on group sums only
                                if kc % 2 == 0:
                                    prev_pt = pt_sb
                                else:
                                    pair = p12.tile([P, ST], f16, tag="pr",
                                                    bufs=5, name="pair")
                                    nc.vector.tensor_tensor(
                                        pair[:], prev_pt[:], pt_sb[:], ALU.add)
                                    if kc % 4 == 1:
                                        prev_pair = pair
                                    else:
                                        quad = p12.tile([P, ST], f16, tag="pr",
                                                        bufs=5, name="quad")
                                        nc.vector.tensor_tensor(
                                            quad[:], prev_pair[:], pair[:],
                                            ALU.add)
                                        nc.tensor.matmul(
                                            l_ps[:], ones_col[:], quad[:],
                                            start=(kc == 3),
                                            stop=(kc == kcs - 1),
                                            skip_group_check=True,
                                        )
                                nc.tensor.matmul(
                                    o_ps[:],
                                    v_all[:, kc, h * HD:(h + 1) * HD],
                                    pt_sb[:],
                                    start=(kc == 0), stop=(kc == kcs - 1),
                                    skip_group_check=True,
                                )
                            recip = p12.tile([1, ST], f32r, tag="rcp", bufs=2,
                                             name="recip")
                            with nc.allow_low_precision(
                                    reason="1/l rounded to f32r for bcast mm"):
                                nc.vector.reciprocal(recip[:], l_ps[:])
                            bc_ps = ps.tile([P, ST], f32, tag="lob", bufs=2,
                                            name="bc_ps")
                            nc.tensor.matmul(
                                bc_ps[:], ones_row[:], recip[:],
                                start=True, stop=True, skip_group_check=True,
                            )
                            # DVE may read only one PSUM operand; stage the
                            # broadcast through SBUF on ACT.
                            bc_sb = p12.tile([P, ST], f32, tag="bcs", bufs=1,
                                             name="bc_sb")
                            nc.scalar.activation(bc_sb[:], bc_ps[:], AF.Copy)
                            at_sb = p12.tile([P, ST], f16, tag="at", bufs=2,
                                             name="at_sb")
                            nc.vector.tensor_tensor(
                                at_sb[:], o_ps[:], bc_sb[:], ALU.mult)
                            if gplan:
                                gi = q2g[q]
                                grp = gplan[gi]
                                qo = grp.index(q) * ST
                                nc.sync.dma_start(
                                    ccg_in[gi][h * P:(h + 1) * P,
                                               qo:qo + ST], at_sb[:])
                                if (not sim_single_core and q == grp[-1]
                                        and h == HPC - 1):
                                    nc.gpsimd.collective_compute(
                                        "AllGather", ALU.bypass,
                                        replica_groups=RG,
                                        ins=[ccg_in[gi][:]],
                                        outs=[ccg_out[gi][:]])
                            elif (gmode == 0 and q < NST - 1) or gmode == 5:
                                nc.sync.dma_start(
                                    cc_in_q[q][h * P:(h + 1) * P, :], at_sb[:])
                                if not sim_single_core and h == HPC - 1:
                                    nc.gpsimd.collective_compute(
                                        "AllGather", ALU.bypass,
                                        replica_groups=RG,
                                        ins=[cc_in_q[q][:]],
                                        outs=[cc_out_q[q][:]])
                            else:
                                jq = q if gmode else 0
                                nc.sync.dma_start(
                                    cc3_in[jq][h // 2][(h % 2) * P:
                                                       (h % 2 + 1) * P, :],
                                    at_sb[:])
                                if not sim_single_core and h % 2 == 1:
                                    nc.gpsimd.collective_compute(
                                        "AllGather", ALU.bypass,
                                        replica_groups=RG,
                                        ins=[cc3_in[jq][h // 2][:]],
                                        outs=[cc3_out[jq][h // 2][:]])

                        # ---------- sim-mode collective stand-in ----------
                        if sim_single_core:
                            zz = p12.tile([P, ST], f16, tag="cc", bufs=2,
                                          name="zz")
                            nc.vector.memset(zz[:], 0.0)
                            if gplan:
                                gi = q2g[q]
                                grp = gplan[gi]
                                if q == grp[-1]:
                                    W = len(grp) * ST
                                    for hh in range(HPC):
                                        tmp = p12.tile([P, W], f16, tag="ccg",
                                                       bufs=2, name="ccg_tmp")
                                        nc.sync.dma_start(
                                            tmp[:],
                                            ccg_in[gi][hh * P:(hh + 1) * P, :])
                                        nc.sync.dma_start(
                                            ccg_out[gi][hh * P:(hh + 1) * P, :],
                                            tmp[:])
                                    zzg = p12.tile([P, W], f16, tag="ccg",
                                                   bufs=2, name="zzg")
                                    nc.vector.memset(zzg[:], 0.0)
                                    for r in range(HPC * P, D, P):
                                        nc.sync.dma_start(
                                            ccg_out[gi][r:r + P, :], zzg[:])
                            elif (gmode == 0 and q < NST - 1) or gmode == 5:
                                for hh in range(HPC):
                                    tmp = p12.tile([P, ST], f16, tag="cc",
                                                   bufs=2, name="cc_tmp")
                                    nc.sync.dma_start(
                                        tmp[:],
                                        cc_in_q[q][hh * P:(hh + 1) * P, :])
                                    nc.sync.dma_start(
                                        cc_out_q[q][hh * P:(hh + 1) * P, :],
                                        tmp[:])
                                for r in range(HPC * P, D, P):
                                    nc.sync.dma_start(
                                        cc_out_q[q][r:r + P, :], zz[:])
                            else:
                                jq = q if gmode else 0
                                for pc in range(2):
                                    for hh in range(2):
                                        tmp = p12.tile([P, ST], f16, tag="cc",
                                                       bufs=2, name="cc3_tmp")
                                        nc.sync.dma_start(
                                            tmp[:],
                                            cc3_in[jq][pc][hh * P:
                                                           (hh + 1) * P, :])
                                        nc.sync.dma_start(
                                            cc3_out[jq][pc][hh * P:
                                                            (hh + 1) * P, :],
                                            tmp[:])
                                    for r in range(2 * P, D // 2, P):
                                        nc.sync.dma_start(
                                            cc3_out[jq][pc][r:r + P, :], zz[:])

                        # ---------- phase 3 (out rows of quarter q) ----------
                        if gplan:
                            gi = q2g[q]
                            grp = gplan[gi]
                            if q == grp[-1]:
                                last_grp = (gi == len(gplan) - 1)
                                for qq in grp:
                                    qo = grp.index(qq) * ST
                                    aq = p12.tile([P, NDC, ST], f16, tag="aq",
                                                  bufs=2, name="aqg")
                                    nc.sync.dma_start(
                                        aq[:],
                                        ccg_out[gi][:, qo:qo + ST].rearrange(
                                            "(o p) f -> p o f", p=P))
                                    for st in range(4 * qq, 4 * qq + 4):
                                        c0 = (st % 4) * P
                                        if last_grp and qq == grp[-1]:
                                            tag = ("o3" if st % 4 in (0, 3)
                                                   else "lob")
                                        else:
                                            tag = "o3"
                                        o3_ps = ps.tile(
                                            [P, ST], f32, tag=tag,
                                            bufs=(1 if tag == "o3" else 2),
                                            name="o3_psg")
                                        for dc in range(NDC):
                                            nc.tensor.matmul(
                                                o3_ps[:], aq[:, dc, c0:c0 + P],
                                                wo_sb[:, dc, :],
                                                start=(dc == 0),
                                                stop=(dc == NDC - 1),
                                                skip_group_check=True,
                                            )
                                        o3_sb = p12.tile([P, ST], f32,
                                                         tag="o3s", bufs=2,
                                                         name="o3_sbg")
                                        nc.vector.tensor_copy(
                                            o3_sb[:], o3_ps[:])
                                        nc.sync.dma_start(
                                            out[st * P:(st + 1) * P, :],
                                            o3_sb[:])
                        elif (gmode == 0 and q < NST - 1) or gmode == 5:
                            # single descriptor-efficient reload of the whole
                            # gathered attn^T for this quarter (rows are
                            # 1KB-contiguous).
                            aq = p12.tile([P, NDC, ST], f16, tag="aq", bufs=1,
                                          name="aq")
                            nc.sync.dma_start(
                                aq[:],
                                cc_out_q[q][:].rearrange("(o p) f -> p o f",
                                                         p=P))
                            for st in range(4 * q, 4 * q + 4):
                                c0 = (st % 4) * P
                                o3_ps = ps.tile([P, ST], f32, tag="o3", bufs=1,
                                                name="o3_ps")
                                for dc in range(NDC):
                                    nc.tensor.matmul(
                                        o3_ps[:], aq[:, dc, c0:c0 + P],
                                        wo_sb[:, dc, :],
                                        start=(dc == 0), stop=(dc == NDC - 1),
                                        skip_group_check=True,
                                    )
                                o3_sb = p12.tile([P, ST], f32, tag="o3s",
                                                 bufs=2, name="o3_sb")
                                nc.vector.tensor_copy(o3_sb[:], o3_ps[:])
                                nc.sync.dma_start(
                                    out[st * P:(st + 1) * P, :], o3_sb[:])
                        else:
                            # last quarter: two 2-head gathers; chunks from
                            # the first pair run while the second gather is
                            # in flight. Two extra PSUM banks via the (now
                            # idle) lob tag let 3 out-tiles accumulate
                            # concurrently across the gather waits.
                            jq = q if gmode else 0
                            a3 = [p12.tile([P, 8, ST], f16, tag="a3", bufs=2,
                                           name=f"a3_{pc}") for pc in range(2)]
                            for pc in range(2):
                                nc.sync.dma_start(
                                    a3[pc][:],
                                    cc3_out[jq][pc][:].rearrange(
                                        "(o p) f -> p o f", p=P))
                            for st in range(4 * q, 4 * q + 4):
                                c0 = (st % 4) * P
                                # banks: o3(1) + lob(2); st3 reuses o3's
                                if q == NST - 1:
                                    tag = "o3" if st % 4 in (0, 3) else "lob"
                                else:
                                    tag = "o3"
                                o3_ps = ps.tile([P, ST], f32,
                                                tag=tag,
                                                bufs=(1 if tag == "o3" else 2),
                                                name="o3_ps3")
                                n_i = 0
                                for pc in range(2):   # gather order
                                    for r in range(4):
                                        for j in range(2):
                                            # head 4r+2pc+j is at rows
                                            # (2r+j)*128 of cc3_out[pc]
                                            dc = 4 * r + 2 * pc + j
                                            nc.tensor.matmul(
                                                o3_ps[:],
                                                a3[pc][:, 2 * r + j,
                                                       c0:c0 + P],
                                                wo_sb[:, dc, :],
                                                start=(n_i == 0),
                                                stop=(n_i == NDC - 1),
                                                skip_group_check=True,
                                            )
                                            n_i += 1
                                o3_sb = p12.tile([P, ST], f32, tag="o3s",
                                                 bufs=2, name="o3_sb3")
                                nc.vector.tensor_copy(o3_sb[:], o3_ps[:])
                                nc.sync.dma_start(
                                    out[st * P:(st + 1) * P, :], o3_sb[:])

                for _rep in range(reps):
                    emit_rep()

    nc.compile()
    return nc


def _get_nc(sim_single_core: bool = False) -> bass.Bass:
    key = bool(sim_single_core)
    if key not in _NC_CACHE:
        _NC_CACHE[key] = build(sim_single_core, gmode=GMODE)
    return _NC_CACHE[key]


def make_core_inputs(x, freqs_cos, freqs_sin, mask, w_in, w_out):
    """Host-side sharding/layout prep. Returns list of 8 per-core input dicts."""
    x = np.asarray(x, np.float32)
    freqs_cos = np.asarray(freqs_cos, np.float32)
    freqs_sin = np.asarray(freqs_sin, np.float32)
    mask = np.asarray(mask, np.float32)
    w_in = np.asarray(w_in, np.float32)
    w_out = np.asarray(w_out, np.float32)

    perm = np.concatenate([np.arange(0, HD, 2), np.arange(1, HD, 2)])
    cs_lo = np.ascontiguousarray(freqs_cos.T).astype(np.float16)   # [64, S]
    sn_ng = np.ascontiguousarray(-freqs_sin.T).astype(np.float16)  # [64, S]
    # {0,1} f16 mask in transposed [key, 4, query] layout: 1 where visible
    m01 = (mask[:ST, :ST].T == 0.0).astype(np.float16)
    mb = np.ascontiguousarray(
        m01.reshape(4, P, ST).transpose(1, 0, 2))                  # [128,4,512]
    xt_b = [np.ascontiguousarray(x[b].T).astype(np.float16) for b in range(B)]
    wo_T = np.ascontiguousarray(w_out.T)                           # [D, D]

    scale = 1.0 / math.sqrt(HD)
    in_maps = []
    for c in range(NC_TOTAL):
        b, g = c // TPG, c % TPG
        heads = range(g * HPC, (g + 1) * HPC)
        wq = np.vstack([w_in[h * HD:(h + 1) * HD][perm] for h in heads]) * scale
        wk = np.vstack([w_in[D + h * HD:D + (h + 1) * HD][perm] for h in heads])
        wv = np.vstack([w_in[2 * D + h * HD:2 * D + (h + 1) * HD] for h in heads])
        in_maps.append({
            "xt": xt_b[b],
            "wq_t": np.ascontiguousarray(wq.T).astype(np.float16),
            "wk_t": np.ascontiguousarray(wk.T).astype(np.float16),
            "wv_t": np.ascontiguousarray(wv.T).astype(np.float16),
            "wo_t": np.ascontiguousarray(wo_T[:, g * ST:(g + 1) * ST]).astype(np.float16),
            "cs_lo": cs_lo,
            "sn_ng": sn_ng,
            "mb": mb,
        })
    return in_maps


def run_spmd(inputs: dict, trace: bool = False):
    """Compile+run on cores 0-7. Returns (full_output, BassKernelResults)."""
    from concourse.bass_utils import run_bass_kernel_spmd

    in_maps = make_core_inputs(**inputs)
    nc = _get_nc(False)
    res = run_bass_kernel_spmd(nc, in_maps, list(range(NC_TOTAL)), trace=trace)
    out_full = np.empty((B, S, D), np.float32)
    for c in range(NC_TOTAL):
        b, g = c // TPG, c % TPG
        out_full[b, :, g * ST:(g + 1) * ST] = res.results[c]["out"]
    return out_full, res


def kernel(x, freqs_cos, freqs_sin, mask, w_in, w_out):
    out, _ = run_spmd(
        dict(x=x, freqs_cos=freqs_cos, freqs_sin=freqs_sin, mask=mask,
             w_in=w_in, w_out=w_out))
    return out
